# revision 5
# baseline (speedup 1.0000x reference)
"""GAT layer (4 heads, 128 dim) on 8 Trainium2 NeuronCores.

Strategy (edge-parallel over dst, TWO independent window systems):
  - Nodes are split into two DRAM K|V tables (L and H) of n_pad/2 rows each
    so row indices fit the int16 index format of the batched dma_gather
    (InstDMAGatherAnt) instruction.  GPSIMD descriptor generation
    (~8.3ns/idx) is the hardware bottleneck, so total gather-index count
    is what matters: each system independently sorts every core's dst
    nodes by its own in-degree (edges whose src lies in that table), so
    per-window slot schedules are tight for BOTH systems (~3% padding,
    vs ~21% for a single shared dst ordering with a lo/hi split).
  - dst nodes are dealt to cores once (round-robin by total degree), so
    both systems agree on which core owns a dst; per-core DRAM scratch
    carries the H partials to the L pass.
  - Phase 1 builds the K|V tables (bf16, 512B rows, biases folded out
    algebraically) with large batched DMAs.
  - Phase H processes the H-system windows and writes raw per-dst partial
    sums (agg[128] | ssum[4]) into a 768B-row DRAM scratch in H order.
  - Phase L processes the L-system windows, gathers the 128 matching H
    partial rows per window (one extra 128-index gather), combines,
    removes pad-slot softmax contributions via a host-side pad-count
    table, applies the V-bias correction, normalizes, and computes
    out = relu(aggn @ Wo^T + bo).
  - Biases enter algebraically: q.(k+bk) = q.k + q.bk (per-head score bias
    inside the exp activation), and sum(exp*(v+bv)) = sum(exp*v) +
    sum(exp)*bv (post-correction).  The reference's global-max shift
    cancels in the normalization up to ~1e-8.
  - Message aggregation over slots uses a log-tree of contiguous adds
    (the strided tensor_reduce is ~5x slower on DVE).
  - No collectives: each core owns a disjoint slice of output rows; the
    host scatters per-core outputs back through the permutation.
"""

import os
import sys

for _p in ("/opt/trn_rl_repo", "/opt/pypackages"):
    if _p not in sys.path:
        sys.path.append(_p)

import numpy as np
import ml_dtypes

P = 128
N_CORES = 8
DIM = 128
NUM_HEADS = 4
HEAD_DIM = 32
INV_SQRT_HD = 1.0 / np.sqrt(HEAD_DIM).astype(np.float32)
EPS = 1e-8
PH1_CHUNK = 1792   # nodes per phase-1 x-chunk (divides 25088 evenly)
HROW = 192         # f32 elements per H-partial row (768B, only 132 used)

_PROGRAM_CACHE = {}


def _build_program(n_pad, n_c, d_schedL, d_schedH, c_idxL, c_idxH):
    import concourse.bass as bass
    import concourse.bacc as bacc
    import concourse.mybir as mybir
    from concourse.tile import TileContext
    from concourse.masks import make_identity

    f32 = mybir.dt.float32
    bf16 = mybir.dt.bfloat16
    i16 = mybir.dt.int16
    n_w = n_c // P
    half = n_pad // 2
    d_max = max(max(d_schedL), max(d_schedH))

    nc = bacc.Bacc()
    xT_full = nc.dram_tensor("xT_full", [P, n_pad], bf16, kind="ExternalInput")
    xT_qL = nc.dram_tensor("xT_qL", [P, n_c], bf16, kind="ExternalInput")
    xT_qH = nc.dram_tensor("xT_qH", [P, n_c], bf16, kind="ExternalInput")
    w_qT = nc.dram_tensor("w_qT", [P, DIM], bf16, kind="ExternalInput")
    w_kT = nc.dram_tensor("w_kT", [P, DIM], bf16, kind="ExternalInput")
    w_vT = nc.dram_tensor("w_vT", [P, DIM], bf16, kind="ExternalInput")
    w_oT = nc.dram_tensor("w_oT", [P, DIM], bf16, kind="ExternalInput")
    b_q = nc.dram_tensor("b_q", [P, DIM], f32, kind="ExternalInput")
    bk_s = nc.dram_tensor("bk_s", [P, DIM], f32, kind="ExternalInput")  # bk*scale
    b_v = nc.dram_tensor("b_v", [P, DIM], f32, kind="ExternalInput")
    b_o = nc.dram_tensor("b_o", [P, DIM], f32, kind="ExternalInput")
    idxL_t = nc.dram_tensor("idxL_t", [P, max(c_idxL, 8)], i16, kind="ExternalInput")
    idxH_t = nc.dram_tensor("idxH_t", [P, max(c_idxH, 8)], i16, kind="ExternalInput")
    hp_idx_t = nc.dram_tensor("hp_idx_t", [P, n_w * 8], i16, kind="ExternalInput")
    npad_t = nc.dram_tensor("npad_t", [P, n_w], f32, kind="ExternalInput")
    out = nc.dram_tensor("out", [n_c, DIM], f32, kind="ExternalOutput")
    kvtabL = nc.dram_tensor("kvtabL", [half + 1, 2 * DIM], bf16)
    kvtabH = nc.dram_tensor("kvtabH", [half + 1, 2 * DIM], bf16)
    hpart = nc.dram_tensor("hpart", [n_c, HROW], f32)

    with TileContext(nc) as tc:
        with (
            tc.tile_pool(name="consts", bufs=1) as cp,
            tc.tile_pool(name="ph1", bufs=2) as p1,
            tc.tile_pool(name="ph1ps", bufs=2, space="PSUM") as p1ps,
            tc.tile_pool(name="kvgp", bufs=3) as kvp,
            tc.tile_pool(name="win", bufs=2) as wp,
            tc.tile_pool(name="hpp", bufs=2) as hpp,
            tc.tile_pool(name="winps", bufs=1, space="PSUM") as pp,
        ):
            # ---- constants ----
            wq_sb = cp.tile([P, DIM], bf16, tag="wq")
            wk_sb = cp.tile([P, DIM], bf16, tag="wk")
            wv_sb = cp.tile([P, DIM], bf16, tag="wv")
            wo_sb = cp.tile([P, DIM], bf16, tag="wo")
            bq_sb = cp.tile([P, DIM], f32, tag="bq")
            bks_sb = cp.tile([P, DIM], f32, tag="bks")
            bv_sb = cp.tile([P, DIM], f32, tag="bv")
            bo_sb = cp.tile([P, DIM], f32, tag="bo")
            nc.sync.dma_start(out=wq_sb[:], in_=w_qT[:])
            nc.sync.dma_start(out=wk_sb[:], in_=w_kT[:])
            nc.sync.dma_start(out=wv_sb[:], in_=w_vT[:])
            nc.sync.dma_start(out=wo_sb[:], in_=w_oT[:])
            nc.sync.dma_start(out=bq_sb[:], in_=b_q[:])
            nc.sync.dma_start(out=bks_sb[:], in_=bk_s[:])
            nc.sync.dma_start(out=bv_sb[:], in_=b_v[:])
            nc.sync.dma_start(out=bo_sb[:], in_=b_o[:])
            idxL_sb = cp.tile([P, max(c_idxL, 8)], i16, tag="idxL")
            idxH_sb = cp.tile([P, max(c_idxH, 8)], i16, tag="idxH")
            hp_idx_sb = cp.tile([P, n_w * 8], i16, tag="hpidx")
            npad_sb = cp.tile([P, n_w], f32, tag="npad")
            nc.sync.dma_start(out=idxL_sb[:], in_=idxL_t[:])
            nc.sync.dma_start(out=idxH_sb[:], in_=idxH_t[:])
            nc.sync.dma_start(out=hp_idx_sb[:], in_=hp_idx_t[:])
            nc.sync.dma_start(out=npad_sb[:], in_=npad_t[:])
            xqL_sb = cp.tile([P, n_c], bf16, tag="xqL")
            xqH_sb = cp.tile([P, n_c], bf16, tag="xqH")
            nc.sync.dma_start(out=xqL_sb[:], in_=xT_qL[:])
            nc.sync.dma_start(out=xqH_sb[:], in_=xT_qH[:])
            ident = cp.tile([P, P], f32, tag="ident")
            make_identity(nc, ident[:])
            zrow = cp.tile([P, 2 * DIM], bf16, tag="zrow")
            nc.vector.memset(zrow[:], 0.0)

            # ---- phase 1: K|V tables (biases folded out) ----
            ng = PH1_CHUNK // P
            for c0 in range(0, n_pad, PH1_CHUNK):
                xc = p1.tile([P, PH1_CHUNK], bf16, tag="xc")
                nc.sync.dma_start(out=xc[:], in_=xT_full[:, c0:c0 + PH1_CHUNK])
                kv_sb = p1.tile([P, ng, 2 * DIM], bf16, tag="kvsb")
                for g in range(ng):
                    ps_kv = p1ps.tile([P, 2 * DIM], f32, tag="pskv")
                    lhs = xc[:, g * P:(g + 1) * P]
                    nc.tensor.matmul(out=ps_kv[:, 0:DIM], lhsT=lhs,
                                     rhs=wk_sb[:], start=True, stop=True)
                    nc.tensor.matmul(out=ps_kv[:, DIM:2 * DIM], lhsT=lhs,
                                     rhs=wv_sb[:], start=True, stop=True)
                    nc.scalar.copy(out=kv_sb[:, g, :], in_=ps_kv[:])
                tab = kvtabL if c0 < half else kvtabH
                r0 = c0 if c0 < half else c0 - half
                nc.sync.dma_start(
                    out=tab[r0:r0 + PH1_CHUNK, :]
                        .rearrange("(g p) e -> p g e", p=P),
                    in_=kv_sb[:])
            # all-zero pad rows
            nc.sync.dma_start(out=kvtabL[half:half + 1, :], in_=zrow[0:1, :])
            nc.sync.dma_start(out=kvtabH[half:half + 1, :], in_=zrow[0:1, :])

            tc.strict_bb_all_engine_barrier()

            def window(sys_is_h, w, d, icol):
                """Shared per-window compute."""
                xq_sb = xqH_sb if sys_is_h else xqL_sb
                idx_sb = idxH_sb if sys_is_h else idxL_sb
                tab = kvtabH if sys_is_h else kvtabL
                row0 = w * P

                # gather K|V rows for this window's slots
                if d > 0:
                    kv_g = kvp.tile([P, d_max, 2 * DIM], bf16, tag="kvg")
                    nc.gpsimd.dma_gather(
                        out_ap=kv_g[:, :d, :],
                        in_ap=tab[:],
                        idxs_ap=idx_sb[:, icol:icol + d * 8],
                        num_idxs=d * P,
                        num_idxs_reg=d * P,
                        elem_size=2 * DIM,
                        single_packet=False,
                    )

                # q_w = xq[:, window] @ WqT + bq   (node-major, bf16)
                ps_q = pp.tile([P, DIM], f32, tag="psq")
                nc.tensor.matmul(out=ps_q[:], lhsT=xq_sb[:, row0:row0 + P],
                                 rhs=wq_sb[:], start=True, stop=True)
                q_w = wp.tile([P, DIM], bf16, tag="qw")
                nc.vector.tensor_tensor(out=q_w[:], in0=ps_q[:], in1=bq_sb[:],
                                        op=mybir.AluOpType.add)

                # per-head score bias qbc[p,h] = scale * q . bk
                qbt = wp.tile([P, DIM], f32, tag="qbt")
                nc.vector.tensor_tensor(out=qbt[:], in0=q_w[:], in1=bks_sb[:],
                                        op=mybir.AluOpType.mult)
                qbc = wp.tile([P, NUM_HEADS], f32, tag="qbc")
                nc.vector.tensor_reduce(
                    out=qbc[:],
                    in_=qbt[:].rearrange("p (h e) -> p h e", e=HEAD_DIM),
                    op=mybir.AluOpType.add, axis=mybir.AxisListType.X)

                ssum = wp.tile([P, NUM_HEADS], f32, tag="ssum")
                t0 = wp.tile([P, (d_max + 1) // 2 + 1, DIM], f32, tag="t0")

                if d > 0:
                    kv3 = kv_g[:, :d, :]
                    # scores: per-slot q.k via broadcast-mult + head reduce
                    tmul = wp.tile([P, d_max, DIM], bf16, tag="tmul")
                    t3 = tmul[:, :d, :]
                    nc.vector.tensor_tensor(
                        out=t3, in0=kv3[:, :, 0:DIM],
                        in1=q_w[:, None, :].broadcast_to([P, d, DIM]),
                        op=mybir.AluOpType.mult)
                    scr = wp.tile([P, d_max * NUM_HEADS], f32, tag="scr")
                    nc.vector.tensor_reduce(
                        out=scr[:, :d * NUM_HEADS],
                        in_=tmul[:, :d, :].rearrange(
                            "p s (h e) -> p s h e", h=NUM_HEADS, e=HEAD_DIM),
                        op=mybir.AluOpType.add, axis=mybir.AxisListType.X)

                    # head-major exp with per-head bias: exp(scale*s + qbc_h)
                    exps = wp.tile([P, NUM_HEADS, d_max], f32, tag="exps")
                    scr3 = scr[:, :d * NUM_HEADS].rearrange(
                        "p (s h) -> p h s", h=NUM_HEADS)
                    for h in range(NUM_HEADS):
                        nc.scalar.activation(
                            out=exps[:, h, :d],
                            in_=scr3[:, h, :],
                            func=mybir.ActivationFunctionType.Exp,
                            bias=qbc[:, h:h + 1], scale=float(INV_SQRT_HD))

                    nc.vector.tensor_reduce(
                        out=ssum[:],
                        in_=exps[:, :, :d],
                        op=mybir.AluOpType.add, axis=mybir.AxisListType.X)

                    # messages: V * exp, then log-tree reduce over slots
                    msm = wp.tile([P, d_max, DIM], bf16, tag="msm")
                    m4 = msm[:, :d, :].rearrange(
                        "p s (h e) -> p s h e", h=NUM_HEADS, e=HEAD_DIM)
                    nc.vector.tensor_tensor(
                        out=m4,
                        in0=kv3[:, :, DIM:2 * DIM].rearrange(
                            "p s (h e) -> p s h e", e=HEAD_DIM),
                        in1=exps[:].rearrange("p h s -> p s h")
                            [:, :d, :, None]
                            .broadcast_to([P, d, NUM_HEADS, HEAD_DIM]),
                        op=mybir.AluOpType.mult)

                    cur = d
                    if cur == 1:
                        nc.scalar.copy(out=t0[:, 0, :], in_=msm[:, 0, :])
                    else:
                        e = cur // 2
                        nc.vector.tensor_tensor(
                            out=t0[:, :e, :], in0=msm[:, 0:e, :],
                            in1=msm[:, e:2 * e, :], op=mybir.AluOpType.add)
                        if cur % 2:
                            nc.scalar.copy(out=t0[:, e, :],
                                           in_=msm[:, 2 * e, :])
                            cur = e + 1
                        else:
                            cur = e
                        while cur > 1:
                            e = cur // 2
                            nc.vector.tensor_tensor(
                                out=t0[:, :e, :], in0=t0[:, :e, :],
                                in1=t0[:, e:2 * e, :], op=mybir.AluOpType.add)
                            if cur % 2:
                                nc.vector.tensor_tensor(
                                    out=t0[:, 0, :], in0=t0[:, 0, :],
                                    in1=t0[:, 2 * e, :],
                                    op=mybir.AluOpType.add)
                            cur = e
                else:
                    nc.vector.memset(t0[:, 0, :], 0.0)
                    nc.vector.memset(ssum[:], 0.0)

                if sys_is_h:
                    # raw partial out: [agg | ssum] (row tail stays garbage)
                    nc.sync.dma_start(
                        out=hpart[row0:row0 + P, 0:DIM], in_=t0[:, 0, :])
                    nc.sync.dma_start(
                        out=hpart[row0:row0 + P, DIM:DIM + NUM_HEADS],
                        in_=ssum[:])
                    return

                # ---- L system: combine with H partial ----
                hp = hpp.tile([P, 1, HROW], f32, tag="hp")
                nc.gpsimd.dma_gather(
                    out_ap=hp[:],
                    in_ap=hpart[:],
                    idxs_ap=hp_idx_sb[:, w * 8:(w + 1) * 8],
                    num_idxs=P,
                    num_idxs_reg=P,
                    elem_size=HROW,
                    single_packet=False,
                )
                agg = wp.tile([P, DIM], f32, tag="agg")
                nc.vector.tensor_tensor(out=agg[:], in0=t0[:, 0, :],
                                        in1=hp[:, 0, 0:DIM],
                                        op=mybir.AluOpType.add)
                sst = wp.tile([P, NUM_HEADS], f32, tag="sst")
                nc.vector.tensor_tensor(out=sst[:], in0=ssum[:],
                                        in1=hp[:, 0, DIM:DIM + NUM_HEADS],
                                        op=mybir.AluOpType.add)

                # remove pad-slot contributions: each pad adds exp(qbc_h)
                eqb = wp.tile([P, NUM_HEADS], f32, tag="eqb")
                nc.scalar.activation(out=eqb[:], in_=qbc[:],
                                     func=mybir.ActivationFunctionType.Exp)
                nc.vector.tensor_scalar(
                    out=eqb[:], in0=eqb[:],
                    scalar1=npad_sb[:, w:w + 1], scalar2=None,
                    op0=mybir.AluOpType.mult)
                nc.vector.tensor_tensor(out=sst[:], in0=sst[:], in1=eqb[:],
                                        op=mybir.AluOpType.subtract)

                # V-bias correction: agg += sst (x) bv
                bvc = wp.tile([P, DIM], f32, tag="bvc")
                nc.vector.tensor_tensor(
                    out=bvc[:].rearrange("p (h e) -> p h e", e=HEAD_DIM),
                    in0=bv_sb[:].rearrange("p (h e) -> p h e", e=HEAD_DIM),
                    in1=sst[:, :, None].broadcast_to([P, NUM_HEADS, HEAD_DIM]),
                    op=mybir.AluOpType.mult)
                nc.vector.tensor_tensor(out=agg[:], in0=agg[:], in1=bvc[:],
                                        op=mybir.AluOpType.add)

                # normalize: agg / (sst + eps), per head
                inv4 = wp.tile([P, NUM_HEADS], f32, tag="inv4")
                nc.vector.tensor_scalar(
                    out=inv4[:], in0=sst[:], scalar1=float(EPS), scalar2=None,
                    op0=mybir.AluOpType.add)
                nc.vector.reciprocal(out=inv4[:], in_=inv4[:])
                aggn = wp.tile([P, DIM], f32, tag="aggn")
                nc.vector.tensor_tensor(
                    out=aggn[:].rearrange("p (h e) -> p h e", e=HEAD_DIM),
                    in0=agg[:].rearrange("p (h e) -> p h e", e=HEAD_DIM),
                    in1=inv4[:, :, None].broadcast_to([P, NUM_HEADS, HEAD_DIM]),
                    op=mybir.AluOpType.mult)

                # out = relu(aggn @ WoT + bo)
                ps_t = pp.tile([P, DIM], f32, tag="pst")
                nc.tensor.transpose(out=ps_t[:], in_=aggn[:], identity=ident[:])
                aggT = wp.tile([P, DIM], bf16, tag="aggT")
                nc.scalar.copy(out=aggT[:], in_=ps_t[:])
                ps_o = pp.tile([P, DIM], f32, tag="pso")
                nc.tensor.matmul(out=ps_o[:], lhsT=aggT[:], rhs=wo_sb[:],
                                 start=True, stop=True)
                res = wp.tile([P, DIM], f32, tag="res")
                nc.vector.tensor_tensor(out=res[:], in0=ps_o[:], in1=bo_sb[:],
                                        op=mybir.AluOpType.add)
                res2 = wp.tile([P, DIM], f32, tag="res2")
                nc.scalar.activation(out=res2[:], in_=res[:],
                                     func=mybir.ActivationFunctionType.Relu)
                nc.sync.dma_start(out=out[row0:row0 + P, :], in_=res2[:])

            # ---- phase H ----
            icol = 0
            for w in range(n_w):
                window(True, w, d_schedH[w], icol)
                icol += d_schedH[w] * 8

            tc.strict_bb_all_engine_barrier()

            # ---- phase L (+ combine) ----
            icol = 0
            for w in range(n_w):
                window(False, w, d_schedL[w], icol)
                icol += d_schedL[w] * 8

    return nc


def prepare(x, edge_index, Wq, bq, Wk, bk, Wv, bv, Wo, bo):
    """Host-side layout prep: permutations, dealing, slot tables. No math."""
    n = x.shape[0]
    n_c = -(-n // (N_CORES * P)) * P
    n_pad = N_CORES * n_c
    n_w = n_c // P
    half = n_pad // 2

    src = np.asarray(edge_index[0], dtype=np.int64)
    dst = np.asarray(edge_index[1], dtype=np.int64)

    # balanced deterministic split into tables 0 (L) / 1 (H)
    rng = np.random.default_rng(12345)
    tperm = rng.permutation(n_pad)
    tab_of = np.empty(n_pad, dtype=np.int8)
    tab_of[tperm[:half]] = 0
    tab_of[tperm[half:]] = 1
    trow = np.empty(n_pad, dtype=np.int64)
    trow[tperm[:half]] = np.arange(half)
    trow[tperm[half:]] = np.arange(half)

    e_tab = tab_of[src]
    deg = np.empty((2, n_pad), dtype=np.int64)
    deg[0] = np.bincount(dst[e_tab == 0], minlength=n_pad)
    deg[1] = np.bincount(dst[e_tab == 1], minlength=n_pad)

    # deal dsts to cores once, round-robin by total degree
    dtot = deg[0] + deg[1]
    order_t = np.argsort(-dtot, kind='stable')
    core_of = np.empty(n_pad, dtype=np.int64)
    core_of[order_t] = np.arange(n_pad) % N_CORES

    # per system: per-core sort by system degree -> window/partition
    node_at = []            # [sys][core, pos] -> node
    pos_of = []             # [sys][node] -> pos within its core
    d_sched = []
    for s in (0, 1):
        na = np.empty((N_CORES, n_c), dtype=np.int64)
        po = np.empty(n_pad, dtype=np.int64)
        for m in range(N_CORES):
            nodes = order_t[m::N_CORES]           # this core's dsts
            o = nodes[np.argsort(-deg[s][nodes], kind='stable')]
            na[m] = o
            po[o] = np.arange(n_c)
        node_at.append(na)
        pos_of.append(po)
        dg = deg[s][na].reshape(N_CORES, n_w, P)
        d_sched.append(tuple(int(v) for v in dg.max(axis=(0, 2))))

    c_idx = [sum(ds) * 8 for ds in d_sched]

    # idx tables per system: [core, 128, cols] int16 (16-wrapped, tiled)
    idx_tabs = []
    for s in (0, 1):
        cols = max(c_idx[s], 8)
        tabs = np.full((N_CORES, 16, cols), np.int16(half), dtype=np.int16)
        esel = e_tab == s
        dst_s = dst[esel]
        src_s = src[esel]
        eo = np.argsort(dst_s, kind='stable')
        dst_o = dst_s[eo]
        src_o = src_s[eo]
        starts = np.zeros(n_pad + 1, dtype=np.int64)
        np.cumsum(deg[s], out=starts[1:])
        slot = np.arange(dst_o.size) - starts[dst_o]

        m = core_of[dst_o]
        posn = pos_of[s][dst_o]
        w_arr = posn // P
        p_arr = posn % P

        blk_off = np.zeros(n_w, dtype=np.int64)
        acc = 0
        for w in range(n_w):
            blk_off[w] = acc
            acc += d_sched[s][w] * 8

        j_g = slot * P + p_arr
        col = blk_off[w_arr] + j_g // 16
        row = j_g % 16
        val = trow[src_o].astype(np.int16)
        flat = tabs.reshape(N_CORES, -1)
        flat[m, row * cols + col] = val
        idx_tabs.append(np.tile(tabs, (1, 8, 1)))

    # hp_idx: for each L window/partition, the H position of that dst
    hp_idx = np.zeros((N_CORES, 16, n_w * 8), dtype=np.int16)
    for m in range(N_CORES):
        hpos = pos_of[1][node_at[0][m]]           # [n_c], < n_c = 6272
        j = np.arange(n_c)                        # j = w*128 + p
        w_arr = j // P
        p_arr = j % P
        col = w_arr * 8 + p_arr // 16
        row = p_arr % 16
        hp_idx[m, row, col] = hpos.astype(np.int16)
    hp_idx = np.tile(hp_idx, (1, 8, 1))

    # pad counts per (core, p, w), in L order: padL + padH for that dst
    dsL = np.asarray(d_sched[0], dtype=np.int64)
    dsH = np.asarray(d_sched[1], dtype=np.int64)
    npad = np.empty((N_CORES, P, n_w), dtype=np.float32)
    for m in range(N_CORES):
        nodes = node_at[0][m].reshape(n_w, P)
        padL = dsL[:, None] - deg[0][nodes]
        wH = pos_of[1][nodes] // P
        padH = dsH[wH] - deg[1][nodes]
        npad[m] = (padL + padH).T.astype(np.float32)

    xpad = np.zeros((n_pad, DIM), dtype=np.float32)
    xpad[:n] = np.asarray(x, dtype=np.float32)
    # xT_full columns in table layout: col j<half -> L row j; else H row
    colnode = np.concatenate([tperm[:half], tperm[half:]])
    xT_full = np.ascontiguousarray(xpad[colnode].T).astype(ml_dtypes.bfloat16)

    in_maps = []
    common = {
        "xT_full": xT_full,
        "w_qT": np.ascontiguousarray(np.asarray(Wq, np.float32).T).astype(ml_dtypes.bfloat16),
        "w_kT": np.ascontiguousarray(np.asarray(Wk, np.float32).T).astype(ml_dtypes.bfloat16),
        "w_vT": np.ascontiguousarray(np.asarray(Wv, np.float32).T).astype(ml_dtypes.bfloat16),
        "w_oT": np.ascontiguousarray(np.asarray(Wo, np.float32).T).astype(ml_dtypes.bfloat16),
        "b_q": np.broadcast_to(np.asarray(bq, np.float32), (P, DIM)).copy(),
        "bk_s": np.broadcast_to(np.asarray(bk, np.float32) * INV_SQRT_HD,
                                (P, DIM)).copy(),
        "b_v": np.broadcast_to(np.asarray(bv, np.float32), (P, DIM)).copy(),
        "b_o": np.broadcast_to(np.asarray(bo, np.float32), (P, DIM)).copy(),
    }
    for m in range(N_CORES):
        im = dict(common)
        im["xT_qL"] = np.ascontiguousarray(xpad[node_at[0][m]].T).astype(ml_dtypes.bfloat16)
        im["xT_qH"] = np.ascontiguousarray(xpad[node_at[1][m]].T).astype(ml_dtypes.bfloat16)
        im["idxL_t"] = idx_tabs[0][m]
        im["idxH_t"] = idx_tabs[1][m]
        im["hp_idx_t"] = hp_idx[m]
        im["npad_t"] = npad[m]
        in_maps.append(im)

    cfg = dict(n=n, n_pad=n_pad, n_c=n_c,
               d_schedL=d_sched[0], d_schedH=d_sched[1],
               c_idxL=c_idx[0], c_idxH=c_idx[1], node_at_L=node_at[0])
    return in_maps, cfg


def get_program(cfg, finalize=True):
    key = (cfg["n_pad"], cfg["n_c"], cfg["d_schedL"], cfg["d_schedH"])
    if key not in _PROGRAM_CACHE:
        nc = _build_program(cfg["n_pad"], cfg["n_c"], cfg["d_schedL"],
                            cfg["d_schedH"], cfg["c_idxL"], cfg["c_idxH"])
        if finalize:
            nc.finalize()
        _PROGRAM_CACHE[key] = nc
    return _PROGRAM_CACHE[key]


def assemble(results, cfg):
    n = cfg["n"]
    out_full = np.empty((n, DIM), dtype=np.float32)
    for m in range(N_CORES):
        nodes = cfg["node_at_L"][m]
        valid = nodes < n
        out_full[nodes[valid]] = np.asarray(results[m]["out"])[valid]
    return out_full


LAST_RESULT = None


def kernel(**inputs):
    global LAST_RESULT
    from concourse.bass_utils import run_bass_kernel_spmd

    in_maps, cfg = prepare(**inputs)
    nc = get_program(cfg)
    res = run_bass_kernel_spmd(nc, in_maps, core_ids=list(range(N_CORES)))
    LAST_RESULT = res
    return assemble(res.results, cfg)


# revision 9
# speedup vs baseline: 1.1438x; 1.1438x over previous
"""GAT layer (4 heads, 128 dim) on 8 Trainium2 NeuronCores.

Strategy (edge-parallel over dst, TWO independent window systems):
  - Nodes are split into two DRAM K|V tables (L and H) of n_pad/2 rows each
    so row indices fit the int16 index format of the batched dma_gather
    (InstDMAGatherAnt) instruction.  GPSIMD descriptor generation
    (~8.3ns/idx) is the hardware bottleneck, so total gather-index count
    is what matters: each system independently sorts every core's dst
    nodes by its own in-degree (edges whose src lies in that table), so
    per-window slot schedules are tight for BOTH systems (~3% padding,
    vs ~21% for a single shared dst ordering with a lo/hi split).
  - dst nodes are dealt to cores once (round-robin by total degree), so
    both systems agree on which core owns a dst; per-core DRAM scratch
    carries the H partials to the L pass.
  - Phase 1 builds the K|V tables (bf16, 512B rows, biases folded out
    algebraically) with large batched DMAs.
  - Phase H processes the H-system windows and writes raw per-dst partial
    sums (agg[128] | ssum[4]) into a 768B-row DRAM scratch in H order.
  - Phase L processes the L-system windows, gathers the 128 matching H
    partial rows per window (one extra 128-index gather), combines,
    removes pad-slot softmax contributions via a host-side pad-count
    table, applies the V-bias correction, normalizes, and computes
    out = relu(aggn @ Wo^T + bo).
  - Biases enter algebraically: q.(k+bk) = q.k + q.bk (per-head score bias
    inside the exp activation), and sum(exp*(v+bv)) = sum(exp*v) +
    sum(exp)*bv (post-correction).  The reference's global-max shift
    cancels in the normalization up to ~1e-8.
  - Message aggregation over slots uses a log-tree of contiguous adds
    (the strided tensor_reduce is ~5x slower on DVE).
  - No collectives: each core owns a disjoint slice of output rows; the
    host scatters per-core outputs back through the permutation.
"""

import os
import sys

for _p in ("/opt/trn_rl_repo", "/opt/pypackages"):
    if _p not in sys.path:
        sys.path.append(_p)

import numpy as np
import ml_dtypes

P = 128
N_CORES = 8
DIM = 128
NUM_HEADS = 4
HEAD_DIM = 32
INV_SQRT_HD = 1.0 / np.sqrt(HEAD_DIM).astype(np.float32)
EPS = 1e-8
PH1_CHUNK = 1792   # nodes per phase-1 x-chunk (divides 25088 evenly)
HROW = 192         # f32 elements per H-partial row (768B, only 132 used)

_PROGRAM_CACHE = {}


def _build_program(n_pad, n_c, d_schedL, d_schedH, c_idxL, c_idxH):
    import concourse.bass as bass
    import concourse.bacc as bacc
    import concourse.mybir as mybir
    from concourse.tile import TileContext
    from concourse.masks import make_identity

    f32 = mybir.dt.float32
    bf16 = mybir.dt.bfloat16
    i16 = mybir.dt.int16
    n_w = n_c // P
    half = n_pad // 2
    d_max = max(max(d_schedL), max(d_schedH))

    nc = bacc.Bacc()
    xT_full = nc.dram_tensor("xT_full", [P, n_pad], bf16, kind="ExternalInput")
    xT_qL = nc.dram_tensor("xT_qL", [P, n_c], bf16, kind="ExternalInput")
    xT_qH = nc.dram_tensor("xT_qH", [P, n_c], bf16, kind="ExternalInput")
    w_qT = nc.dram_tensor("w_qT", [P, DIM], bf16, kind="ExternalInput")
    w_kT = nc.dram_tensor("w_kT", [P, DIM], bf16, kind="ExternalInput")
    w_vT = nc.dram_tensor("w_vT", [P, DIM], bf16, kind="ExternalInput")
    w_oT = nc.dram_tensor("w_oT", [P, DIM], bf16, kind="ExternalInput")
    b_q = nc.dram_tensor("b_q", [P, DIM], f32, kind="ExternalInput")
    bk_s = nc.dram_tensor("bk_s", [P, DIM], f32, kind="ExternalInput")  # bk*scale
    b_v = nc.dram_tensor("b_v", [P, DIM], f32, kind="ExternalInput")
    b_o = nc.dram_tensor("b_o", [P, DIM], f32, kind="ExternalInput")
    idxL_t = nc.dram_tensor("idxL_t", [P, max(c_idxL, 8)], i16, kind="ExternalInput")
    idxH_t = nc.dram_tensor("idxH_t", [P, max(c_idxH, 8)], i16, kind="ExternalInput")
    hp_idx_t = nc.dram_tensor("hp_idx_t", [P, n_w * 8], i16, kind="ExternalInput")
    npad_t = nc.dram_tensor("npad_t", [P, n_w], f32, kind="ExternalInput")
    out = nc.dram_tensor("out", [n_c, DIM], f32, kind="ExternalOutput")
    kvtabL = nc.dram_tensor("kvtabL", [half + 1, 2 * DIM], bf16)
    kvtabH = nc.dram_tensor("kvtabH", [half + 1, 2 * DIM], bf16)
    hpart = nc.dram_tensor("hpart", [n_c, HROW], f32)

    with TileContext(nc) as tc:
        with (
            tc.tile_pool(name="consts", bufs=1) as cp,
            tc.tile_pool(name="ph1", bufs=3) as p1,
            tc.tile_pool(name="ph1ps", bufs=3, space="PSUM") as p1ps,
            tc.tile_pool(name="kvgp", bufs=3) as kvp,
            tc.tile_pool(name="win", bufs=2) as wp,
            tc.tile_pool(name="hpp", bufs=2) as hpp,
            tc.tile_pool(name="winps", bufs=1, space="PSUM") as pp,
        ):
            # ---- constants ----
            wq_sb = cp.tile([P, DIM], bf16, tag="wq")
            wk_sb = cp.tile([P, DIM], bf16, tag="wk")
            wv_sb = cp.tile([P, DIM], bf16, tag="wv")
            wo_sb = cp.tile([P, DIM], bf16, tag="wo")
            bq_sb = cp.tile([P, DIM], f32, tag="bq")
            bks_sb = cp.tile([P, DIM], f32, tag="bks")
            bv_sb = cp.tile([P, DIM], f32, tag="bv")
            bo_sb = cp.tile([P, DIM], f32, tag="bo")
            nc.sync.dma_start(out=wq_sb[:], in_=w_qT[:])
            nc.sync.dma_start(out=wk_sb[:], in_=w_kT[:])
            nc.sync.dma_start(out=wv_sb[:], in_=w_vT[:])
            nc.sync.dma_start(out=wo_sb[:], in_=w_oT[:])
            nc.sync.dma_start(out=bq_sb[:], in_=b_q[:])
            nc.sync.dma_start(out=bks_sb[:], in_=bk_s[:])
            nc.sync.dma_start(out=bv_sb[:], in_=b_v[:])
            nc.sync.dma_start(out=bo_sb[:], in_=b_o[:])
            idxL_sb = cp.tile([P, max(c_idxL, 8)], i16, tag="idxL")
            idxH_sb = cp.tile([P, max(c_idxH, 8)], i16, tag="idxH")
            hp_idx_sb = cp.tile([P, n_w * 8], i16, tag="hpidx")
            npad_sb = cp.tile([P, n_w], f32, tag="npad")
            nc.sync.dma_start(out=idxL_sb[:], in_=idxL_t[:])
            nc.sync.dma_start(out=idxH_sb[:], in_=idxH_t[:])
            nc.sync.dma_start(out=hp_idx_sb[:], in_=hp_idx_t[:])
            nc.sync.dma_start(out=npad_sb[:], in_=npad_t[:])
            xqL_sb = cp.tile([P, n_c], bf16, tag="xqL")
            xqH_sb = cp.tile([P, n_c], bf16, tag="xqH")
            nc.sync.dma_start(out=xqL_sb[:], in_=xT_qL[:])
            nc.sync.dma_start(out=xqH_sb[:], in_=xT_qH[:])
            ident = cp.tile([P, P], f32, tag="ident")
            make_identity(nc, ident[:])
            zrow = cp.tile([P, 2 * DIM], bf16, tag="zrow")
            nc.vector.memset(zrow[:], 0.0)
            eps4 = cp.tile([P, NUM_HEADS], f32, tag="eps4")
            nc.vector.memset(eps4[:], float(EPS))

            # ---- phase 1: K|V tables (biases folded out) ----
            ng = PH1_CHUNK // P
            for c0 in range(0, n_pad, PH1_CHUNK):
                xc = p1.tile([P, PH1_CHUNK], bf16, tag="xc")
                nc.sync.dma_start(out=xc[:], in_=xT_full[:, c0:c0 + PH1_CHUNK])
                kv_sb = p1.tile([P, ng, 2 * DIM], bf16, tag="kvsb")
                for g in range(ng):
                    ps_kv = p1ps.tile([P, 2 * DIM], f32, tag="pskv")
                    lhs = xc[:, g * P:(g + 1) * P]
                    nc.tensor.matmul(out=ps_kv[:, 0:DIM], lhsT=lhs,
                                     rhs=wk_sb[:], start=True, stop=True)
                    nc.tensor.matmul(out=ps_kv[:, DIM:2 * DIM], lhsT=lhs,
                                     rhs=wv_sb[:], start=True, stop=True)
                    nc.scalar.copy(out=kv_sb[:, g, :], in_=ps_kv[:])
                tab = kvtabL if c0 < half else kvtabH
                r0 = c0 if c0 < half else c0 - half
                nc.sync.dma_start(
                    out=tab[r0:r0 + PH1_CHUNK, :]
                        .rearrange("(g p) e -> p g e", p=P),
                    in_=kv_sb[:])
            # all-zero pad rows
            nc.sync.dma_start(out=kvtabL[half:half + 1, :], in_=zrow[0:1, :])
            nc.sync.dma_start(out=kvtabH[half:half + 1, :], in_=zrow[0:1, :])

            tc.strict_bb_all_engine_barrier()

            def window(sys_is_h, w, d, icol):
                """Shared per-window compute."""
                xq_sb = xqH_sb if sys_is_h else xqL_sb
                idx_sb = idxH_sb if sys_is_h else idxL_sb
                tab = kvtabH if sys_is_h else kvtabL
                row0 = w * P

                # gather K|V rows for this window's slots
                if d > 0:
                    kv_g = kvp.tile([P, d_max, 2 * DIM], bf16, tag="kvg")
                    nc.gpsimd.dma_gather(
                        out_ap=kv_g[:, :d, :],
                        in_ap=tab[:],
                        idxs_ap=idx_sb[:, icol:icol + d * 8],
                        num_idxs=d * P,
                        num_idxs_reg=d * P,
                        elem_size=2 * DIM,
                        single_packet=False,
                    )

                # q_w = xq[:, window] @ WqT + bq   (node-major, bf16)
                ps_q = pp.tile([P, DIM], f32, tag="psq")
                nc.tensor.matmul(out=ps_q[:], lhsT=xq_sb[:, row0:row0 + P],
                                 rhs=wq_sb[:], start=True, stop=True)
                q_w = wp.tile([P, DIM], bf16, tag="qw")
                nc.vector.tensor_tensor(out=q_w[:], in0=ps_q[:], in1=bq_sb[:],
                                        op=mybir.AluOpType.add)

                # per-head score bias qbc[p,h] = scale * q . bk
                qbt = wp.tile([P, DIM], f32, tag="qbt")
                nc.vector.tensor_tensor(out=qbt[:], in0=q_w[:], in1=bks_sb[:],
                                        op=mybir.AluOpType.mult)
                qbc = wp.tile([P, NUM_HEADS], f32, tag="qbc")
                nc.vector.tensor_reduce(
                    out=qbc[:],
                    in_=qbt[:].rearrange("p (h e) -> p h e", e=HEAD_DIM),
                    op=mybir.AluOpType.add, axis=mybir.AxisListType.X)

                ssum = wp.tile([P, NUM_HEADS], f32, tag="ssum")
                t0 = wp.tile([P, (d_max + 1) // 2 + 1, DIM], f32, tag="t0")

                if d > 0:
                    kv3 = kv_g[:, :d, :]
                    # scores: per-slot q.k via broadcast-mult + head reduce
                    tmul = wp.tile([P, d_max, DIM], bf16, tag="tmul")
                    t3 = tmul[:, :d, :]
                    nc.vector.tensor_tensor(
                        out=t3, in0=kv3[:, :, 0:DIM],
                        in1=q_w[:, None, :].broadcast_to([P, d, DIM]),
                        op=mybir.AluOpType.mult)
                    scr = wp.tile([P, d_max * NUM_HEADS], f32, tag="scr")
                    nc.vector.tensor_reduce(
                        out=scr[:, :d * NUM_HEADS],
                        in_=tmul[:, :d, :].rearrange(
                            "p s (h e) -> p s h e", h=NUM_HEADS, e=HEAD_DIM),
                        op=mybir.AluOpType.add, axis=mybir.AxisListType.X)

                    # head-major exp with per-head bias: exp(scale*s + qbc_h)
                    exps = wp.tile([P, NUM_HEADS, d_max], f32, tag="exps")
                    scr3 = scr[:, :d * NUM_HEADS].rearrange(
                        "p (s h) -> p h s", h=NUM_HEADS)
                    for h in range(NUM_HEADS):
                        nc.scalar.activation(
                            out=exps[:, h, :d],
                            in_=scr3[:, h, :],
                            func=mybir.ActivationFunctionType.Exp,
                            bias=qbc[:, h:h + 1], scale=float(INV_SQRT_HD))

                    nc.vector.tensor_reduce(
                        out=ssum[:],
                        in_=exps[:, :, :d],
                        op=mybir.AluOpType.add, axis=mybir.AxisListType.X)

                    # messages: V * exp, then log-tree reduce over slots
                    msm = wp.tile([P, d_max, DIM], bf16, tag="msm")
                    m4 = msm[:, :d, :].rearrange(
                        "p s (h e) -> p s h e", h=NUM_HEADS, e=HEAD_DIM)
                    nc.vector.tensor_tensor(
                        out=m4,
                        in0=kv3[:, :, DIM:2 * DIM].rearrange(
                            "p s (h e) -> p s h e", e=HEAD_DIM),
                        in1=exps[:].rearrange("p h s -> p s h")
                            [:, :d, :, None]
                            .broadcast_to([P, d, NUM_HEADS, HEAD_DIM]),
                        op=mybir.AluOpType.mult)

                    cur = d
                    if cur == 1:
                        nc.scalar.copy(out=t0[:, 0, :], in_=msm[:, 0, :])
                    else:
                        e = cur // 2
                        nc.vector.tensor_tensor(
                            out=t0[:, :e, :], in0=msm[:, 0:e, :],
                            in1=msm[:, e:2 * e, :], op=mybir.AluOpType.add)
                        if cur % 2:
                            nc.scalar.copy(out=t0[:, e, :],
                                           in_=msm[:, 2 * e, :])
                            cur = e + 1
                        else:
                            cur = e
                        while cur > 1:
                            e = cur // 2
                            nc.vector.tensor_tensor(
                                out=t0[:, :e, :], in0=t0[:, :e, :],
                                in1=t0[:, e:2 * e, :], op=mybir.AluOpType.add)
                            if cur % 2:
                                nc.vector.tensor_tensor(
                                    out=t0[:, 0, :], in0=t0[:, 0, :],
                                    in1=t0[:, 2 * e, :],
                                    op=mybir.AluOpType.add)
                            cur = e
                else:
                    nc.vector.memset(t0[:, 0, :], 0.0)
                    nc.vector.memset(ssum[:], 0.0)

                if sys_is_h:
                    # raw partial out: [agg | ssum] (row tail stays garbage)
                    nc.sync.dma_start(
                        out=hpart[row0:row0 + P, 0:DIM], in_=t0[:, 0, :])
                    nc.sync.dma_start(
                        out=hpart[row0:row0 + P, DIM:DIM + NUM_HEADS],
                        in_=ssum[:])
                    return

                # ---- L system: combine with H partial ----
                hp = hpp.tile([P, 1, HROW], f32, tag="hp")
                nc.gpsimd.dma_gather(
                    out_ap=hp[:],
                    in_ap=hpart[:],
                    idxs_ap=hp_idx_sb[:, w * 8:(w + 1) * 8],
                    num_idxs=P,
                    num_idxs_reg=P,
                    elem_size=HROW,
                    single_packet=False,
                )
                agg = wp.tile([P, DIM], f32, tag="agg")
                nc.vector.tensor_tensor(out=agg[:], in0=t0[:, 0, :],
                                        in1=hp[:, 0, 0:DIM],
                                        op=mybir.AluOpType.add)
                sst = wp.tile([P, NUM_HEADS], f32, tag="sst")
                nc.vector.tensor_tensor(out=sst[:], in0=ssum[:],
                                        in1=hp[:, 0, DIM:DIM + NUM_HEADS],
                                        op=mybir.AluOpType.add)

                # remove pad-slot contributions: each pad adds exp(qbc_h)
                eqb = wp.tile([P, NUM_HEADS], f32, tag="eqb")
                nc.scalar.activation(out=eqb[:], in_=qbc[:],
                                     func=mybir.ActivationFunctionType.Exp)
                nc.vector.tensor_tensor(
                    out=eqb[:], in0=eqb[:],
                    in1=npad_sb[:, w:w + 1].broadcast_to([P, NUM_HEADS]),
                    op=mybir.AluOpType.mult)
                nc.vector.tensor_tensor(out=sst[:], in0=sst[:], in1=eqb[:],
                                        op=mybir.AluOpType.subtract)

                # V-bias correction: agg += sst (x) bv
                bvc = wp.tile([P, DIM], f32, tag="bvc")
                nc.vector.tensor_tensor(
                    out=bvc[:].rearrange("p (h e) -> p h e", e=HEAD_DIM),
                    in0=bv_sb[:].rearrange("p (h e) -> p h e", e=HEAD_DIM),
                    in1=sst[:, :, None].broadcast_to([P, NUM_HEADS, HEAD_DIM]),
                    op=mybir.AluOpType.mult)
                nc.vector.tensor_tensor(out=agg[:], in0=agg[:], in1=bvc[:],
                                        op=mybir.AluOpType.add)

                # normalize: agg / (sst + eps), per head
                inv4 = wp.tile([P, NUM_HEADS], f32, tag="inv4")
                nc.vector.tensor_tensor(out=inv4[:], in0=sst[:], in1=eps4[:],
                                        op=mybir.AluOpType.add)
                nc.vector.reciprocal(out=inv4[:], in_=inv4[:])
                aggn = wp.tile([P, DIM], f32, tag="aggn")
                nc.vector.tensor_tensor(
                    out=aggn[:].rearrange("p (h e) -> p h e", e=HEAD_DIM),
                    in0=agg[:].rearrange("p (h e) -> p h e", e=HEAD_DIM),
                    in1=inv4[:, :, None].broadcast_to([P, NUM_HEADS, HEAD_DIM]),
                    op=mybir.AluOpType.mult)

                # out = relu(aggn @ WoT + bo)
                ps_t = pp.tile([P, DIM], f32, tag="pst")
                nc.tensor.transpose(out=ps_t[:], in_=aggn[:], identity=ident[:])
                aggT = wp.tile([P, DIM], bf16, tag="aggT")
                nc.scalar.copy(out=aggT[:], in_=ps_t[:])
                ps_o = pp.tile([P, DIM], f32, tag="pso")
                nc.tensor.matmul(out=ps_o[:], lhsT=aggT[:], rhs=wo_sb[:],
                                 start=True, stop=True)
                res = wp.tile([P, DIM], f32, tag="res")
                nc.vector.tensor_tensor(out=res[:], in0=ps_o[:], in1=bo_sb[:],
                                        op=mybir.AluOpType.add)
                res2 = wp.tile([P, DIM], f32, tag="res2")
                nc.scalar.activation(out=res2[:], in_=res[:],
                                     func=mybir.ActivationFunctionType.Relu)
                nc.sync.dma_start(out=out[row0:row0 + P, :], in_=res2[:])

            # ---- phase H ----
            icol = 0
            for w in range(n_w):
                window(True, w, d_schedH[w], icol)
                icol += d_schedH[w] * 8

            tc.strict_bb_all_engine_barrier()

            # ---- phase L (+ combine) ----
            icol = 0
            for w in range(n_w):
                window(False, w, d_schedL[w], icol)
                icol += d_schedL[w] * 8

    return nc


def prepare(x, edge_index, Wq, bq, Wk, bk, Wv, bv, Wo, bo):
    """Host-side layout prep: permutations, dealing, slot tables. No math."""
    n = x.shape[0]
    n_c = -(-n // (N_CORES * P)) * P
    n_pad = N_CORES * n_c
    n_w = n_c // P
    half = n_pad // 2

    src = np.asarray(edge_index[0], dtype=np.int64)
    dst = np.asarray(edge_index[1], dtype=np.int64)

    # balanced deterministic split into tables 0 (L) / 1 (H)
    rng = np.random.default_rng(12345)
    tperm = rng.permutation(n_pad)
    tab_of = np.empty(n_pad, dtype=np.int8)
    tab_of[tperm[:half]] = 0
    tab_of[tperm[half:]] = 1
    trow = np.empty(n_pad, dtype=np.int64)
    trow[tperm[:half]] = np.arange(half)
    trow[tperm[half:]] = np.arange(half)

    e_tab = tab_of[src]
    deg = np.empty((2, n_pad), dtype=np.int64)
    deg[0] = np.bincount(dst[e_tab == 0], minlength=n_pad)
    deg[1] = np.bincount(dst[e_tab == 1], minlength=n_pad)

    # deal dsts to cores once, round-robin by total degree
    dtot = deg[0] + deg[1]
    order_t = np.argsort(-dtot, kind='stable')
    core_of = np.empty(n_pad, dtype=np.int64)
    core_of[order_t] = np.arange(n_pad) % N_CORES

    # per system: per-core sort by system degree -> window/partition
    node_at = []            # [sys][core, pos] -> node
    pos_of = []             # [sys][node] -> pos within its core
    d_sched = []
    for s in (0, 1):
        na = np.empty((N_CORES, n_c), dtype=np.int64)
        po = np.empty(n_pad, dtype=np.int64)
        for m in range(N_CORES):
            nodes = order_t[m::N_CORES]           # this core's dsts
            o = nodes[np.argsort(-deg[s][nodes], kind='stable')]
            na[m] = o
            po[o] = np.arange(n_c)
        node_at.append(na)
        pos_of.append(po)
        dg = deg[s][na].reshape(N_CORES, n_w, P)
        d_sched.append(tuple(int(v) for v in dg.max(axis=(0, 2))))

    c_idx = [sum(ds) * 8 for ds in d_sched]

    # idx tables per system: [core, 128, cols] int16 (16-wrapped, tiled)
    idx_tabs = []
    for s in (0, 1):
        cols = max(c_idx[s], 8)
        tabs = np.full((N_CORES, 16, cols), np.int16(half), dtype=np.int16)
        esel = e_tab == s
        dst_s = dst[esel]
        src_s = src[esel]
        eo = np.argsort(dst_s, kind='stable')
        dst_o = dst_s[eo]
        src_o = src_s[eo]
        starts = np.zeros(n_pad + 1, dtype=np.int64)
        np.cumsum(deg[s], out=starts[1:])
        slot = np.arange(dst_o.size) - starts[dst_o]

        m = core_of[dst_o]
        posn = pos_of[s][dst_o]
        w_arr = posn // P
        p_arr = posn % P

        blk_off = np.zeros(n_w, dtype=np.int64)
        acc = 0
        for w in range(n_w):
            blk_off[w] = acc
            acc += d_sched[s][w] * 8

        j_g = slot * P + p_arr
        col = blk_off[w_arr] + j_g // 16
        row = j_g % 16
        val = trow[src_o].astype(np.int16)
        flat = tabs.reshape(N_CORES, -1)
        flat[m, row * cols + col] = val
        idx_tabs.append(np.tile(tabs, (1, 8, 1)))

    # hp_idx: for each L window/partition, the H position of that dst
    hp_idx = np.zeros((N_CORES, 16, n_w * 8), dtype=np.int16)
    for m in range(N_CORES):
        hpos = pos_of[1][node_at[0][m]]           # [n_c], < n_c = 6272
        j = np.arange(n_c)                        # j = w*128 + p
        w_arr = j // P
        p_arr = j % P
        col = w_arr * 8 + p_arr // 16
        row = p_arr % 16
        hp_idx[m, row, col] = hpos.astype(np.int16)
    hp_idx = np.tile(hp_idx, (1, 8, 1))

    # pad counts per (core, p, w), in L order: padL + padH for that dst
    dsL = np.asarray(d_sched[0], dtype=np.int64)
    dsH = np.asarray(d_sched[1], dtype=np.int64)
    npad = np.empty((N_CORES, P, n_w), dtype=np.float32)
    for m in range(N_CORES):
        nodes = node_at[0][m].reshape(n_w, P)
        padL = dsL[:, None] - deg[0][nodes]
        wH = pos_of[1][nodes] // P
        padH = dsH[wH] - deg[1][nodes]
        npad[m] = (padL + padH).T.astype(np.float32)

    xpad = np.zeros((n_pad, DIM), dtype=np.float32)
    xpad[:n] = np.asarray(x, dtype=np.float32)
    # xT_full columns in table layout: col j<half -> L row j; else H row
    colnode = np.concatenate([tperm[:half], tperm[half:]])
    xT_full = np.ascontiguousarray(xpad[colnode].T).astype(ml_dtypes.bfloat16)

    in_maps = []
    common = {
        "xT_full": xT_full,
        "w_qT": np.ascontiguousarray(np.asarray(Wq, np.float32).T).astype(ml_dtypes.bfloat16),
        "w_kT": np.ascontiguousarray(np.asarray(Wk, np.float32).T).astype(ml_dtypes.bfloat16),
        "w_vT": np.ascontiguousarray(np.asarray(Wv, np.float32).T).astype(ml_dtypes.bfloat16),
        "w_oT": np.ascontiguousarray(np.asarray(Wo, np.float32).T).astype(ml_dtypes.bfloat16),
        "b_q": np.broadcast_to(np.asarray(bq, np.float32), (P, DIM)).copy(),
        "bk_s": np.broadcast_to(np.asarray(bk, np.float32) * INV_SQRT_HD,
                                (P, DIM)).copy(),
        "b_v": np.broadcast_to(np.asarray(bv, np.float32), (P, DIM)).copy(),
        "b_o": np.broadcast_to(np.asarray(bo, np.float32), (P, DIM)).copy(),
    }
    for m in range(N_CORES):
        im = dict(common)
        im["xT_qL"] = np.ascontiguousarray(xpad[node_at[0][m]].T).astype(ml_dtypes.bfloat16)
        im["xT_qH"] = np.ascontiguousarray(xpad[node_at[1][m]].T).astype(ml_dtypes.bfloat16)
        im["idxL_t"] = idx_tabs[0][m]
        im["idxH_t"] = idx_tabs[1][m]
        im["hp_idx_t"] = hp_idx[m]
        im["npad_t"] = npad[m]
        in_maps.append(im)

    cfg = dict(n=n, n_pad=n_pad, n_c=n_c,
               d_schedL=d_sched[0], d_schedH=d_sched[1],
               c_idxL=c_idx[0], c_idxH=c_idx[1], node_at_L=node_at[0])
    return in_maps, cfg


def get_program(cfg, finalize=True):
    key = (cfg["n_pad"], cfg["n_c"], cfg["d_schedL"], cfg["d_schedH"])
    if key not in _PROGRAM_CACHE:
        nc = _build_program(cfg["n_pad"], cfg["n_c"], cfg["d_schedL"],
                            cfg["d_schedH"], cfg["c_idxL"], cfg["c_idxH"])
        if finalize:
            nc.finalize()
        _PROGRAM_CACHE[key] = nc
    return _PROGRAM_CACHE[key]


def assemble(results, cfg):
    n = cfg["n"]
    out_full = np.empty((n, DIM), dtype=np.float32)
    for m in range(N_CORES):
        nodes = cfg["node_at_L"][m]
        valid = nodes < n
        out_full[nodes[valid]] = np.asarray(results[m]["out"])[valid]
    return out_full


LAST_RESULT = None


def kernel(**inputs):
    global LAST_RESULT
    from concourse.bass_utils import run_bass_kernel_spmd

    in_maps, cfg = prepare(**inputs)
    nc = get_program(cfg)
    res = run_bass_kernel_spmd(nc, in_maps, core_ids=list(range(N_CORES)))
    LAST_RESULT = res
    return assemble(res.results, cfg)


# revision 11
# speedup vs baseline: 1.2203x; 1.0669x over previous
"""GAT layer (4 heads, 128 dim) on 8 Trainium2 NeuronCores.

Strategy (edge-parallel over dst, TWO independent window systems):
  - Nodes are split into two DRAM K|V tables (L and H) of n_pad/2 rows each
    so row indices fit the int16 index format of the batched dma_gather
    (InstDMAGatherAnt) instruction.  GPSIMD descriptor generation
    (~8.3ns/idx) is the hardware bottleneck, so total gather-index count
    is what matters: each system independently sorts every core's dst
    nodes by its own in-degree (edges whose src lies in that table), so
    per-window slot schedules are tight for BOTH systems (~3% padding,
    vs ~21% for a single shared dst ordering with a lo/hi split).
  - dst nodes are dealt to cores once (round-robin by total degree), so
    both systems agree on which core owns a dst; per-core DRAM scratch
    carries the H partials to the L pass.
  - Phase 1 builds the K|V tables (bf16, 512B rows, biases folded out
    algebraically) with large batched DMAs.
  - Phase H processes the H-system windows and writes raw per-dst partial
    sums (agg[128] | ssum[4]) into a 768B-row DRAM scratch in H order.
  - Phase L processes the L-system windows, gathers the 128 matching H
    partial rows per window (one extra 128-index gather), combines,
    removes pad-slot softmax contributions via a host-side pad-count
    table, applies the V-bias correction, normalizes, and computes
    out = relu(aggn @ Wo^T + bo).
  - Biases enter algebraically: q.(k+bk) = q.k + q.bk (per-head score bias
    inside the exp activation), and sum(exp*(v+bv)) = sum(exp*v) +
    sum(exp)*bv (post-correction).  The reference's global-max shift
    cancels in the normalization up to ~1e-8.
  - Message aggregation over slots uses a log-tree of contiguous adds
    (the strided tensor_reduce is ~5x slower on DVE).
  - No collectives: each core owns a disjoint slice of output rows; the
    host scatters per-core outputs back through the permutation.
"""

import os
import sys

for _p in ("/opt/trn_rl_repo", "/opt/pypackages"):
    if _p not in sys.path:
        sys.path.append(_p)

import numpy as np
import ml_dtypes

P = 128
N_CORES = 8
DIM = 128
NUM_HEADS = 4
HEAD_DIM = 32
INV_SQRT_HD = 1.0 / np.sqrt(HEAD_DIM).astype(np.float32)
EPS = 1e-8
PH1_CHUNK = 1792   # nodes per phase-1 x-chunk (divides 25088 evenly)
HROW = 192         # f32 elements per H-partial row (768B, only 132 used)

_PROGRAM_CACHE = {}


def _build_program(n_pad, n_c, d_schedL, d_schedH, c_idxL, c_idxH):
    import concourse.bass as bass
    import concourse.bacc as bacc
    import concourse.mybir as mybir
    from concourse.tile import TileContext
    from concourse.masks import make_identity

    f32 = mybir.dt.float32
    bf16 = mybir.dt.bfloat16
    i16 = mybir.dt.int16
    n_w = n_c // P
    half = n_pad // 2
    d_max = max(max(d_schedL), max(d_schedH))

    nc = bacc.Bacc()
    xT_full = nc.dram_tensor("xT_full", [P, n_pad], bf16, kind="ExternalInput")
    xT_qL = nc.dram_tensor("xT_qL", [P, n_c], bf16, kind="ExternalInput")
    xT_qH = nc.dram_tensor("xT_qH", [P, n_c], bf16, kind="ExternalInput")
    w_qT = nc.dram_tensor("w_qT", [P, DIM], bf16, kind="ExternalInput")
    w_kT = nc.dram_tensor("w_kT", [P, DIM], bf16, kind="ExternalInput")
    w_vT = nc.dram_tensor("w_vT", [P, DIM], bf16, kind="ExternalInput")
    w_oT = nc.dram_tensor("w_oT", [P, DIM], bf16, kind="ExternalInput")
    b_q = nc.dram_tensor("b_q", [P, DIM], f32, kind="ExternalInput")
    bk_s = nc.dram_tensor("bk_s", [P, DIM], f32, kind="ExternalInput")  # bk*scale
    b_v = nc.dram_tensor("b_v", [P, DIM], f32, kind="ExternalInput")
    b_o = nc.dram_tensor("b_o", [P, DIM], f32, kind="ExternalInput")
    idxL_t = nc.dram_tensor("idxL_t", [P, max(c_idxL, 8)], i16, kind="ExternalInput")
    idxH_t = nc.dram_tensor("idxH_t", [P, max(c_idxH, 8)], i16, kind="ExternalInput")
    hp_idx_t = nc.dram_tensor("hp_idx_t", [P, n_w * 8], i16, kind="ExternalInput")
    npad_t = nc.dram_tensor("npad_t", [P, n_w], f32, kind="ExternalInput")
    out = nc.dram_tensor("out", [n_c, DIM], f32, kind="ExternalOutput")
    kvtabL = nc.dram_tensor("kvtabL", [half + 1, 2 * DIM], bf16)
    kvtabH = nc.dram_tensor("kvtabH", [half + 1, 2 * DIM], bf16)
    hpart = nc.dram_tensor("hpart", [n_c, HROW], f32)

    with TileContext(nc) as tc:
        with (
            tc.tile_pool(name="consts", bufs=1) as cp,
            tc.tile_pool(name="ph1", bufs=3) as p1,
            tc.tile_pool(name="ph1ps", bufs=3, space="PSUM") as p1ps,
            tc.tile_pool(name="kvgp", bufs=3) as kvp,
            tc.tile_pool(name="win", bufs=2) as wp,
            tc.tile_pool(name="hpp", bufs=2) as hpp,
            tc.tile_pool(name="winps", bufs=1, space="PSUM") as pp,
        ):
            # ---- constants ----
            wq_sb = cp.tile([P, DIM], bf16, tag="wq")
            wk_sb = cp.tile([P, DIM], bf16, tag="wk")
            wv_sb = cp.tile([P, DIM], bf16, tag="wv")
            wo_sb = cp.tile([P, DIM], bf16, tag="wo")
            bq_sb = cp.tile([P, DIM], f32, tag="bq")
            bks_sb = cp.tile([P, DIM], f32, tag="bks")
            bv_sb = cp.tile([P, DIM], f32, tag="bv")
            bo_sb = cp.tile([P, DIM], f32, tag="bo")
            nc.sync.dma_start(out=wq_sb[:], in_=w_qT[:])
            nc.sync.dma_start(out=wk_sb[:], in_=w_kT[:])
            nc.sync.dma_start(out=wv_sb[:], in_=w_vT[:])
            nc.sync.dma_start(out=wo_sb[:], in_=w_oT[:])
            nc.sync.dma_start(out=bq_sb[:], in_=b_q[:])
            nc.sync.dma_start(out=bks_sb[:], in_=bk_s[:])
            nc.sync.dma_start(out=bv_sb[:], in_=b_v[:])
            nc.sync.dma_start(out=bo_sb[:], in_=b_o[:])
            idxL_sb = cp.tile([P, max(c_idxL, 8)], i16, tag="idxL")
            idxH_sb = cp.tile([P, max(c_idxH, 8)], i16, tag="idxH")
            hp_idx_sb = cp.tile([P, n_w * 8], i16, tag="hpidx")
            npad_sb = cp.tile([P, n_w], f32, tag="npad")
            nc.sync.dma_start(out=idxL_sb[:], in_=idxL_t[:])
            nc.sync.dma_start(out=idxH_sb[:], in_=idxH_t[:])
            nc.sync.dma_start(out=hp_idx_sb[:], in_=hp_idx_t[:])
            nc.sync.dma_start(out=npad_sb[:], in_=npad_t[:])
            xqL_sb = cp.tile([P, n_c], bf16, tag="xqL")
            xqH_sb = cp.tile([P, n_c], bf16, tag="xqH")
            nc.sync.dma_start(out=xqL_sb[:], in_=xT_qL[:])
            nc.sync.dma_start(out=xqH_sb[:], in_=xT_qH[:])
            ident = cp.tile([P, P], f32, tag="ident")
            make_identity(nc, ident[:])
            zrow = cp.tile([P, 2 * DIM], bf16, tag="zrow")
            nc.vector.memset(zrow[:], 0.0)
            eps4 = cp.tile([P, NUM_HEADS], f32, tag="eps4")
            nc.vector.memset(eps4[:], float(EPS))

            # ---- phase 1: K|V tables (biases folded out) ----
            # H table first: phase-H gathers only depend on kvtabH, so they
            # start while the L table is still being built (tile tracks the
            # per-tensor DRAM RAW dependencies).
            ng = PH1_CHUNK // P
            chunks = (list(range(half, n_pad, PH1_CHUNK))
                      + list(range(0, half, PH1_CHUNK)))
            for c0 in chunks:
                xc = p1.tile([P, PH1_CHUNK], bf16, tag="xc")
                nc.sync.dma_start(out=xc[:], in_=xT_full[:, c0:c0 + PH1_CHUNK])
                kv_sb = p1.tile([P, ng, 2 * DIM], bf16, tag="kvsb")
                for g in range(ng):
                    ps_kv = p1ps.tile([P, 2 * DIM], f32, tag="pskv")
                    lhs = xc[:, g * P:(g + 1) * P]
                    nc.tensor.matmul(out=ps_kv[:, 0:DIM], lhsT=lhs,
                                     rhs=wk_sb[:], start=True, stop=True)
                    nc.tensor.matmul(out=ps_kv[:, DIM:2 * DIM], lhsT=lhs,
                                     rhs=wv_sb[:], start=True, stop=True)
                    nc.scalar.copy(out=kv_sb[:, g, :], in_=ps_kv[:])
                tab = kvtabL if c0 < half else kvtabH
                r0 = c0 if c0 < half else c0 - half
                nc.sync.dma_start(
                    out=tab[r0:r0 + PH1_CHUNK, :]
                        .rearrange("(g p) e -> p g e", p=P),
                    in_=kv_sb[:])
                if c0 + PH1_CHUNK == n_pad:
                    nc.sync.dma_start(out=kvtabH[half:half + 1, :],
                                      in_=zrow[0:1, :])
            nc.sync.dma_start(out=kvtabL[half:half + 1, :], in_=zrow[0:1, :])

            def window(sys_is_h, w, d, icol):
                """Shared per-window compute."""
                xq_sb = xqH_sb if sys_is_h else xqL_sb
                idx_sb = idxH_sb if sys_is_h else idxL_sb
                tab = kvtabH if sys_is_h else kvtabL
                row0 = w * P

                # gather K|V rows for this window's slots
                if d > 0:
                    kv_g = kvp.tile([P, d_max, 2 * DIM], bf16, tag="kvg")
                    nc.gpsimd.dma_gather(
                        out_ap=kv_g[:, :d, :],
                        in_ap=tab[:],
                        idxs_ap=idx_sb[:, icol:icol + d * 8],
                        num_idxs=d * P,
                        num_idxs_reg=d * P,
                        elem_size=2 * DIM,
                        single_packet=False,
                    )

                # q_w = xq[:, window] @ WqT + bq   (node-major, bf16)
                ps_q = pp.tile([P, DIM], f32, tag="psq")
                nc.tensor.matmul(out=ps_q[:], lhsT=xq_sb[:, row0:row0 + P],
                                 rhs=wq_sb[:], start=True, stop=True)
                q_w = wp.tile([P, DIM], bf16, tag="qw")
                nc.vector.tensor_tensor(out=q_w[:], in0=ps_q[:], in1=bq_sb[:],
                                        op=mybir.AluOpType.add)

                # per-head score bias qbc[p,h] = scale * q . bk
                qbt = wp.tile([P, DIM], f32, tag="qbt")
                nc.vector.tensor_tensor(out=qbt[:], in0=q_w[:], in1=bks_sb[:],
                                        op=mybir.AluOpType.mult)
                qbc = wp.tile([P, NUM_HEADS], f32, tag="qbc")
                nc.vector.tensor_reduce(
                    out=qbc[:],
                    in_=qbt[:].rearrange("p (h e) -> p h e", e=HEAD_DIM),
                    op=mybir.AluOpType.add, axis=mybir.AxisListType.X)

                ssum = wp.tile([P, NUM_HEADS], f32, tag="ssum")
                t0 = wp.tile([P, (d_max + 1) // 2 + 1, DIM], f32, tag="t0")

                if d > 0:
                    kv3 = kv_g[:, :d, :]
                    # scores: per-slot q.k via broadcast-mult + head reduce
                    tmul = wp.tile([P, d_max, DIM], bf16, tag="tmul")
                    t3 = tmul[:, :d, :]
                    nc.vector.tensor_tensor(
                        out=t3, in0=kv3[:, :, 0:DIM],
                        in1=q_w[:, None, :].broadcast_to([P, d, DIM]),
                        op=mybir.AluOpType.mult)
                    scr = wp.tile([P, d_max * NUM_HEADS], f32, tag="scr")
                    nc.vector.tensor_reduce(
                        out=scr[:, :d * NUM_HEADS],
                        in_=tmul[:, :d, :].rearrange(
                            "p s (h e) -> p s h e", h=NUM_HEADS, e=HEAD_DIM),
                        op=mybir.AluOpType.add, axis=mybir.AxisListType.X)

                    # head-major exp with per-head bias: exp(scale*s + qbc_h)
                    exps = wp.tile([P, NUM_HEADS, d_max], f32, tag="exps")
                    scr3 = scr[:, :d * NUM_HEADS].rearrange(
                        "p (s h) -> p h s", h=NUM_HEADS)
                    for h in range(NUM_HEADS):
                        nc.scalar.activation(
                            out=exps[:, h, :d],
                            in_=scr3[:, h, :],
                            func=mybir.ActivationFunctionType.Exp,
                            bias=qbc[:, h:h + 1], scale=float(INV_SQRT_HD))

                    nc.vector.tensor_reduce(
                        out=ssum[:],
                        in_=exps[:, :, :d],
                        op=mybir.AluOpType.add, axis=mybir.AxisListType.X)

                    # messages: V * exp, then log-tree reduce over slots
                    msm = wp.tile([P, d_max, DIM], bf16, tag="msm")
                    m4 = msm[:, :d, :].rearrange(
                        "p s (h e) -> p s h e", h=NUM_HEADS, e=HEAD_DIM)
                    nc.vector.tensor_tensor(
                        out=m4,
                        in0=kv3[:, :, DIM:2 * DIM].rearrange(
                            "p s (h e) -> p s h e", e=HEAD_DIM),
                        in1=exps[:].rearrange("p h s -> p s h")
                            [:, :d, :, None]
                            .broadcast_to([P, d, NUM_HEADS, HEAD_DIM]),
                        op=mybir.AluOpType.mult)

                    cur = d
                    if cur == 1:
                        nc.scalar.copy(out=t0[:, 0, :], in_=msm[:, 0, :])
                    else:
                        e = cur // 2
                        nc.vector.tensor_tensor(
                            out=t0[:, :e, :], in0=msm[:, 0:e, :],
                            in1=msm[:, e:2 * e, :], op=mybir.AluOpType.add)
                        if cur % 2:
                            nc.scalar.copy(out=t0[:, e, :],
                                           in_=msm[:, 2 * e, :])
                            cur = e + 1
                        else:
                            cur = e
                        while cur > 1:
                            e = cur // 2
                            nc.vector.tensor_tensor(
                                out=t0[:, :e, :], in0=t0[:, :e, :],
                                in1=t0[:, e:2 * e, :], op=mybir.AluOpType.add)
                            if cur % 2:
                                nc.vector.tensor_tensor(
                                    out=t0[:, 0, :], in0=t0[:, 0, :],
                                    in1=t0[:, 2 * e, :],
                                    op=mybir.AluOpType.add)
                            cur = e
                else:
                    nc.vector.memset(t0[:, 0, :], 0.0)
                    nc.vector.memset(ssum[:], 0.0)

                if sys_is_h:
                    # raw partial out: [agg | ssum] (row tail stays garbage)
                    nc.sync.dma_start(
                        out=hpart[row0:row0 + P, 0:DIM], in_=t0[:, 0, :])
                    nc.sync.dma_start(
                        out=hpart[row0:row0 + P, DIM:DIM + NUM_HEADS],
                        in_=ssum[:])
                    return

                # ---- L system: combine with H partial ----
                hp = hpp.tile([P, 1, HROW], f32, tag="hp")
                nc.gpsimd.dma_gather(
                    out_ap=hp[:],
                    in_ap=hpart[:],
                    idxs_ap=hp_idx_sb[:, w * 8:(w + 1) * 8],
                    num_idxs=P,
                    num_idxs_reg=P,
                    elem_size=HROW,
                    single_packet=False,
                )
                agg = wp.tile([P, DIM], f32, tag="agg")
                nc.vector.tensor_tensor(out=agg[:], in0=t0[:, 0, :],
                                        in1=hp[:, 0, 0:DIM],
                                        op=mybir.AluOpType.add)
                sst = wp.tile([P, NUM_HEADS], f32, tag="sst")
                nc.vector.tensor_tensor(out=sst[:], in0=ssum[:],
                                        in1=hp[:, 0, DIM:DIM + NUM_HEADS],
                                        op=mybir.AluOpType.add)

                # remove pad-slot contributions: each pad adds exp(qbc_h)
                eqb = wp.tile([P, NUM_HEADS], f32, tag="eqb")
                nc.scalar.activation(out=eqb[:], in_=qbc[:],
                                     func=mybir.ActivationFunctionType.Exp)
                nc.vector.tensor_tensor(
                    out=eqb[:], in0=eqb[:],
                    in1=npad_sb[:, w:w + 1].broadcast_to([P, NUM_HEADS]),
                    op=mybir.AluOpType.mult)
                nc.vector.tensor_tensor(out=sst[:], in0=sst[:], in1=eqb[:],
                                        op=mybir.AluOpType.subtract)

                # V-bias correction: agg += sst (x) bv
                bvc = wp.tile([P, DIM], f32, tag="bvc")
                nc.vector.tensor_tensor(
                    out=bvc[:].rearrange("p (h e) -> p h e", e=HEAD_DIM),
                    in0=bv_sb[:].rearrange("p (h e) -> p h e", e=HEAD_DIM),
                    in1=sst[:, :, None].broadcast_to([P, NUM_HEADS, HEAD_DIM]),
                    op=mybir.AluOpType.mult)
                nc.vector.tensor_tensor(out=agg[:], in0=agg[:], in1=bvc[:],
                                        op=mybir.AluOpType.add)

                # normalize: agg / (sst + eps), per head
                inv4 = wp.tile([P, NUM_HEADS], f32, tag="inv4")
                nc.vector.tensor_tensor(out=inv4[:], in0=sst[:], in1=eps4[:],
                                        op=mybir.AluOpType.add)
                nc.vector.reciprocal(out=inv4[:], in_=inv4[:])
                aggn = wp.tile([P, DIM], f32, tag="aggn")
                nc.vector.tensor_tensor(
                    out=aggn[:].rearrange("p (h e) -> p h e", e=HEAD_DIM),
                    in0=agg[:].rearrange("p (h e) -> p h e", e=HEAD_DIM),
                    in1=inv4[:, :, None].broadcast_to([P, NUM_HEADS, HEAD_DIM]),
                    op=mybir.AluOpType.mult)

                # out = relu(aggn @ WoT + bo)
                ps_t = pp.tile([P, DIM], f32, tag="pst")
                nc.tensor.transpose(out=ps_t[:], in_=aggn[:], identity=ident[:])
                aggT = wp.tile([P, DIM], bf16, tag="aggT")
                nc.scalar.copy(out=aggT[:], in_=ps_t[:])
                ps_o = pp.tile([P, DIM], f32, tag="pso")
                nc.tensor.matmul(out=ps_o[:], lhsT=aggT[:], rhs=wo_sb[:],
                                 start=True, stop=True)
                res = wp.tile([P, DIM], f32, tag="res")
                nc.vector.tensor_tensor(out=res[:], in0=ps_o[:], in1=bo_sb[:],
                                        op=mybir.AluOpType.add)
                res2 = wp.tile([P, DIM], f32, tag="res2")
                nc.scalar.activation(out=res2[:], in_=res[:],
                                     func=mybir.ActivationFunctionType.Relu)
                nc.sync.dma_start(out=out[row0:row0 + P, :], in_=res2[:])

            # ---- phase H ----
            icol = 0
            for w in range(n_w):
                window(True, w, d_schedH[w], icol)
                icol += d_schedH[w] * 8

            # ---- phase L (+ combine) ----
            icol = 0
            for w in range(n_w):
                window(False, w, d_schedL[w], icol)
                icol += d_schedL[w] * 8

    return nc


def prepare(x, edge_index, Wq, bq, Wk, bk, Wv, bv, Wo, bo):
    """Host-side layout prep: permutations, dealing, slot tables. No math."""
    n = x.shape[0]
    n_c = -(-n // (N_CORES * P)) * P
    n_pad = N_CORES * n_c
    n_w = n_c // P
    half = n_pad // 2

    src = np.asarray(edge_index[0], dtype=np.int64)
    dst = np.asarray(edge_index[1], dtype=np.int64)

    # balanced deterministic split into tables 0 (L) / 1 (H)
    rng = np.random.default_rng(12345)
    tperm = rng.permutation(n_pad)
    tab_of = np.empty(n_pad, dtype=np.int8)
    tab_of[tperm[:half]] = 0
    tab_of[tperm[half:]] = 1
    trow = np.empty(n_pad, dtype=np.int64)
    trow[tperm[:half]] = np.arange(half)
    trow[tperm[half:]] = np.arange(half)

    e_tab = tab_of[src]
    deg = np.empty((2, n_pad), dtype=np.int64)
    deg[0] = np.bincount(dst[e_tab == 0], minlength=n_pad)
    deg[1] = np.bincount(dst[e_tab == 1], minlength=n_pad)

    # deal dsts to cores once, round-robin by total degree
    dtot = deg[0] + deg[1]
    order_t = np.argsort(-dtot, kind='stable')
    core_of = np.empty(n_pad, dtype=np.int64)
    core_of[order_t] = np.arange(n_pad) % N_CORES

    # per system: per-core sort by system degree -> window/partition
    node_at = []            # [sys][core, pos] -> node
    pos_of = []             # [sys][node] -> pos within its core
    d_sched = []
    for s in (0, 1):
        na = np.empty((N_CORES, n_c), dtype=np.int64)
        po = np.empty(n_pad, dtype=np.int64)
        for m in range(N_CORES):
            nodes = order_t[m::N_CORES]           # this core's dsts
            o = nodes[np.argsort(-deg[s][nodes], kind='stable')]
            na[m] = o
            po[o] = np.arange(n_c)
        node_at.append(na)
        pos_of.append(po)
        dg = deg[s][na].reshape(N_CORES, n_w, P)
        d_sched.append(tuple(int(v) for v in dg.max(axis=(0, 2))))

    c_idx = [sum(ds) * 8 for ds in d_sched]

    # idx tables per system: [core, 128, cols] int16 (16-wrapped, tiled)
    idx_tabs = []
    for s in (0, 1):
        cols = max(c_idx[s], 8)
        tabs = np.full((N_CORES, 16, cols), np.int16(half), dtype=np.int16)
        esel = e_tab == s
        dst_s = dst[esel]
        src_s = src[esel]
        eo = np.argsort(dst_s, kind='stable')
        dst_o = dst_s[eo]
        src_o = src_s[eo]
        starts = np.zeros(n_pad + 1, dtype=np.int64)
        np.cumsum(deg[s], out=starts[1:])
        slot = np.arange(dst_o.size) - starts[dst_o]

        m = core_of[dst_o]
        posn = pos_of[s][dst_o]
        w_arr = posn // P
        p_arr = posn % P

        blk_off = np.zeros(n_w, dtype=np.int64)
        acc = 0
        for w in range(n_w):
            blk_off[w] = acc
            acc += d_sched[s][w] * 8

        j_g = slot * P + p_arr
        col = blk_off[w_arr] + j_g // 16
        row = j_g % 16
        val = trow[src_o].astype(np.int16)
        flat = tabs.reshape(N_CORES, -1)
        flat[m, row * cols + col] = val
        idx_tabs.append(np.tile(tabs, (1, 8, 1)))

    # hp_idx: for each L window/partition, the H position of that dst
    hp_idx = np.zeros((N_CORES, 16, n_w * 8), dtype=np.int16)
    for m in range(N_CORES):
        hpos = pos_of[1][node_at[0][m]]           # [n_c], < n_c = 6272
        j = np.arange(n_c)                        # j = w*128 + p
        w_arr = j // P
        p_arr = j % P
        col = w_arr * 8 + p_arr // 16
        row = p_arr % 16
        hp_idx[m, row, col] = hpos.astype(np.int16)
    hp_idx = np.tile(hp_idx, (1, 8, 1))

    # pad counts per (core, p, w), in L order: padL + padH for that dst
    dsL = np.asarray(d_sched[0], dtype=np.int64)
    dsH = np.asarray(d_sched[1], dtype=np.int64)
    npad = np.empty((N_CORES, P, n_w), dtype=np.float32)
    for m in range(N_CORES):
        nodes = node_at[0][m].reshape(n_w, P)
        padL = dsL[:, None] - deg[0][nodes]
        wH = pos_of[1][nodes] // P
        padH = dsH[wH] - deg[1][nodes]
        npad[m] = (padL + padH).T.astype(np.float32)

    xpad = np.zeros((n_pad, DIM), dtype=np.float32)
    xpad[:n] = np.asarray(x, dtype=np.float32)
    # xT_full columns in table layout: col j<half -> L row j; else H row
    colnode = np.concatenate([tperm[:half], tperm[half:]])
    xT_full = np.ascontiguousarray(xpad[colnode].T).astype(ml_dtypes.bfloat16)

    in_maps = []
    common = {
        "xT_full": xT_full,
        "w_qT": np.ascontiguousarray(np.asarray(Wq, np.float32).T).astype(ml_dtypes.bfloat16),
        "w_kT": np.ascontiguousarray(np.asarray(Wk, np.float32).T).astype(ml_dtypes.bfloat16),
        "w_vT": np.ascontiguousarray(np.asarray(Wv, np.float32).T).astype(ml_dtypes.bfloat16),
        "w_oT": np.ascontiguousarray(np.asarray(Wo, np.float32).T).astype(ml_dtypes.bfloat16),
        "b_q": np.broadcast_to(np.asarray(bq, np.float32), (P, DIM)).copy(),
        "bk_s": np.broadcast_to(np.asarray(bk, np.float32) * INV_SQRT_HD,
                                (P, DIM)).copy(),
        "b_v": np.broadcast_to(np.asarray(bv, np.float32), (P, DIM)).copy(),
        "b_o": np.broadcast_to(np.asarray(bo, np.float32), (P, DIM)).copy(),
    }
    for m in range(N_CORES):
        im = dict(common)
        im["xT_qL"] = np.ascontiguousarray(xpad[node_at[0][m]].T).astype(ml_dtypes.bfloat16)
        im["xT_qH"] = np.ascontiguousarray(xpad[node_at[1][m]].T).astype(ml_dtypes.bfloat16)
        im["idxL_t"] = idx_tabs[0][m]
        im["idxH_t"] = idx_tabs[1][m]
        im["hp_idx_t"] = hp_idx[m]
        im["npad_t"] = npad[m]
        in_maps.append(im)

    cfg = dict(n=n, n_pad=n_pad, n_c=n_c,
               d_schedL=d_sched[0], d_schedH=d_sched[1],
               c_idxL=c_idx[0], c_idxH=c_idx[1], node_at_L=node_at[0])
    return in_maps, cfg


def get_program(cfg, finalize=True):
    key = (cfg["n_pad"], cfg["n_c"], cfg["d_schedL"], cfg["d_schedH"])
    if key not in _PROGRAM_CACHE:
        nc = _build_program(cfg["n_pad"], cfg["n_c"], cfg["d_schedL"],
                            cfg["d_schedH"], cfg["c_idxL"], cfg["c_idxH"])
        if finalize:
            nc.finalize()
        _PROGRAM_CACHE[key] = nc
    return _PROGRAM_CACHE[key]


def assemble(results, cfg):
    n = cfg["n"]
    out_full = np.empty((n, DIM), dtype=np.float32)
    for m in range(N_CORES):
        nodes = cfg["node_at_L"][m]
        valid = nodes < n
        out_full[nodes[valid]] = np.asarray(results[m]["out"])[valid]
    return out_full


LAST_RESULT = None


def kernel(**inputs):
    global LAST_RESULT
    from concourse.bass_utils import run_bass_kernel_spmd

    in_maps, cfg = prepare(**inputs)
    nc = get_program(cfg)
    res = run_bass_kernel_spmd(nc, in_maps, core_ids=list(range(N_CORES)))
    LAST_RESULT = res
    return assemble(res.results, cfg)
